# revision 17
# baseline (speedup 1.0000x reference)
"""Baseline Trainium2 Bass kernel (30.7us) — kept for wedge-testing/reference."""

import os
import numpy as np

T = 262144
NOUT = T - 8            # 262136 valid conv output positions
NCORES = 8
KLEN = 15
SIGMA = 0.005

EEG_NCOL = 4096         # eeg matmul columns per core (8 outputs each)
EEG_COLS = EEG_NCOL + 1  # phase row length (g=1 needs one extra column)
WAV_NCOL = 2731         # wav matmul columns per core (12 outputs each)
EEG_TC = 8 * EEG_NCOL   # 32768 eeg outputs per core
WAV_TC = 12 * WAV_NCOL  # 32772 wav outputs per core

_NC_CACHE = {}
LAST_RESULT = None      # BassKernelResults of the most recent device run


def _sinc_rows(mu):
    k = np.linspace(-1.0, 1.0, KLEN)
    kk = (k[None, :] - np.asarray(mu, np.float64)[:, None]) / SIGMA
    nos = np.sum(np.abs(kk) < 1e-5, axis=1)
    kk = np.where((nos >= 0.5)[:, None], kk - 5e-5, kk)
    return np.sin(np.pi * kk) / (np.pi * kk)


def _composite_wav_weights(mu, proj_w, conv_w_i):
    krn = _sinc_rows(mu)                                  # [16,15]
    a = np.asarray(proj_w, np.float64)[:, 0, 0]           # [16]
    W = np.asarray(conv_w_i, np.float64)                  # [10,16,9]
    E = np.zeros((10, 23))
    for j in range(9):
        E[:, j:j + 15] += np.einsum('oc,cm->om', W[:, :, j] * a[None, :], krn)
    return E


def _eeg_lhsT(W1):
    W1 = np.asarray(W1, np.float64)
    out = np.zeros((128, 160))
    g, c, r, o, dt = np.meshgrid(np.arange(2), np.arange(16), np.arange(8),
                                 np.arange(10), np.arange(8), indexing='ij')
    j = 8 * g + r - dt
    valid = (j >= 0) & (j < 9)
    out[(c * 8 + r)[valid], (g * 80 + o * 8 + dt)[valid]] = \
        W1[o[valid], c[valid], np.clip(j[valid], 0, 8)]
    return out.astype(np.float32)


def _wav_lhsT(E):
    out = np.zeros((36, 120))
    v, q, o, dt = np.meshgrid(np.arange(12), np.arange(3), np.arange(10),
                              np.arange(12), indexing='ij')
    s = 12 * q + v - dt
    valid = (s >= 0) & (s < 23)
    out[(v * 3 + q)[valid], (o * 12 + dt)[valid]] = E[o[valid], np.clip(s[valid], 0, 22)]
    return out.astype(np.float32)


def _core_starts(k):
    return (min(k * 32767, NOUT - EEG_TC), min(k * 32767, NOUT - WAV_TC))


def _eeg_phases(eeg, k):
    s_e, _ = _core_starts(k)
    v = eeg[:, s_e:s_e + 8 * EEG_COLS]                  # [16, 32776]
    p = v.reshape(16, EEG_COLS, 8).transpose(0, 2, 1)   # [16,8,4097]
    return p.reshape(128, EEG_COLS)


def _wav_phases(w_pad, k):
    _, s_w = _core_starts(k)
    sl = w_pad[s_w:s_w + 12 * (WAV_NCOL + 2)]
    y = sl.reshape(WAV_NCOL + 2, 12).T                  # y[v,m] = sl[12m+v]
    out = np.empty((36, WAV_NCOL), dtype=w_pad.dtype)
    for q in range(3):
        out[q::3, :] = y[:, q:q + WAV_NCOL]
    return out


def _build_nc():
    import concourse.bacc as bacc
    import concourse.tile as tile
    import concourse.mybir as mybir

    f32 = mybir.dt.float32
    f16 = mybir.dt.float16
    nc = bacc.Bacc("TRN2", target_bir_lowering=False, debug=False,
                   num_devices=NCORES)

    eegP = nc.dram_tensor("eegP", [128, EEG_COLS], f16, kind="ExternalInput")
    wavP = nc.dram_tensor("wavP", [36, 2 * WAV_NCOL], f16, kind="ExternalInput")
    wts = nc.dram_tensor("wts", [128, 400], f16, kind="ExternalInput")
    out = nc.dram_tensor("out", [128, 10], f16, kind="ExternalOutput")

    N_ECHUNK = 2                 # eeg input loaded in 2 column chunks
    ECHUNK = 2048                # chunk j covers cols [2048j, 2048j+2049)
    N_WARM = 6                   # dummy matmuls to warm the PE clock gate

    with tile.TileContext(nc) as tc:
        with (
            tc.tile_pool(name="sb", bufs=1) as sb,
            tc.tile_pool(name="ps", bufs=4, space="PSUM") as psp,
        ):
            scr = sb.tile([128, 512], f16, tag="scr")
            nc.gpsimd.memset(scr[:], 0.0)
            wps = psp.tile([120, 1024], f32, tag="ps", name="wps")
            for _ in range(N_WARM):
                nc.tensor.matmul(wps[0:80, 0:512], scr[:, 0:80], scr[:],
                                 start=True, stop=True)

            echunks = [sb.tile([128, ECHUNK + 1], f16, tag=f"eegchunk{j}",
                               name=f"eegchunk{j}") for j in range(N_ECHUNK)]
            wts_t = sb.tile([128, 400], f16, tag="wts")
            wav_t = sb.tile([36, 2 * WAV_NCOL], f16, tag="wav")
            nc.scalar.dma_start(wts_t[:], wts[:])
            nc.sync.dma_start(echunks[0][:], eegP[:, 0:ECHUNK + 1])
            nc.sync.dma_start(echunks[1][:], eegP[:, ECHUNK:2 * ECHUNK + 1])
            nc.scalar.dma_start(wav_t[:], wavP[:])
            wE_t = wts_t[:, 0:160]

            out16 = sb.tile([128, 10], f16, tag="out16")
            nc.gpsimd.memset(out16[:], 0.0)
            mF = sb.tile([120, 4], f32, tag="mF")
            nc.gpsimd.memset(mF[:], 0.0)
            stg = [sb.tile([120, 1024], f16, tag=f"stg{i}", name=f"stg{i}")
                   for i in range(4)]

            X = mybir.AxisListType.X
            Copy = mybir.ActivationFunctionType.Copy

            for p in range(4):
                ch = echunks[p // 2]
                base = (p % 2) * 1024
                ps = psp.tile([120, 1024], f32, tag="ps", name=f"pse{p}")
                for g in range(2):
                    for j in range(2):
                        lo = j * 512
                        nc.tensor.matmul(ps[0:80, lo:lo + 512],
                                         wE_t[:, 80 * g:80 * g + 80],
                                         ch[:, base + lo + g:base + lo + g + 512],
                                         start=(g == 0), stop=(g == 1))
                if p % 2 == 0:
                    nc.vector.reduce_max(mF[0:80, p // 2:p // 2 + 1],
                                         ps[0:80, :], axis=X)
                else:
                    nc.scalar.activation(stg[p // 2][0:80, :], ps[0:80, :], Copy)
            nc.vector.reduce_max(out16[0:80, 4:5], stg[0][0:80, :], axis=X)
            nc.vector.reduce_max(out16[0:80, 5:6], stg[1][0:80, :], axis=X)

            for si in range(2):
                s0, s1 = (stg[2], stg[3]) if si == 0 else (stg[0], stg[1])
                for p in range(3):
                    ps = psp.tile([120, 1024], f32, tag="ps", name=f"psw{si}{p}")
                    for j in range(2):
                        n0 = si * WAV_NCOL + (2 * p + j) * 512
                        nn = min(512, (si + 1) * WAV_NCOL - n0)
                        nc.tensor.matmul(ps[:, j * 512:j * 512 + nn],
                                         wts_t[0:36, 160 + 120 * si:280 + 120 * si],
                                         wav_t[:, n0:n0 + nn],
                                         start=True, stop=True)
                    if p < 2:
                        nc.scalar.activation((s0 if p == 0 else s1)[:], ps[:], Copy)
                    else:
                        nc.vector.reduce_max(mF[:, 2 + si:3 + si],
                                             ps[:, 0:683], axis=X)
                nc.vector.reduce_max(out16[0:120, 6 + 2 * si:7 + 2 * si],
                                     s0[:], axis=X)
                nc.vector.reduce_max(out16[0:120, 7 + 2 * si:8 + 2 * si],
                                     s1[:], axis=X)

            nc.vector.tensor_copy(out16[0:120, 0:4], mF[:])

            nc.sync.dma_start(out[:], out16[:])

    nc.compile()
    return nc


def _get_nc():
    if "nc" not in _NC_CACHE:
        _NC_CACHE["nc"] = _build_nc()
    return _NC_CACHE["nc"]


def _prepare_in_maps(x, mu, projA_w, projB_w, conv_w):
    x = np.asarray(x, np.float32)
    eeg = np.ascontiguousarray(x[0, 0, 1:17, :]).astype(np.float16)
    zt = np.zeros(64, np.float32)
    w_padA = np.concatenate([np.zeros(7, np.float32), x[0, 0, 0, :], zt]
                            ).astype(np.float16)
    w_padB = np.concatenate([np.zeros(7, np.float32), x[0, 0, 17, :], zt]
                            ).astype(np.float16)

    conv_w = np.asarray(conv_w)
    E_A = _composite_wav_weights(mu, projA_w, conv_w[0])
    E_B = _composite_wav_weights(mu, projB_w, conv_w[2])
    wts_np = np.zeros((128, 400), np.float16)
    wts_np[:, 0:160] = _eeg_lhsT(conv_w[1])
    wts_np[0:36, 160:280] = _wav_lhsT(E_A)
    wts_np[0:36, 280:400] = _wav_lhsT(E_B)

    in_maps = []
    for k in range(NCORES):
        wavp = np.concatenate([_wav_phases(w_padA, k), _wav_phases(w_padB, k)],
                              axis=1)
        in_maps.append({
            "eegP": np.ascontiguousarray(_eeg_phases(eeg, k)),
            "wavP": np.ascontiguousarray(wavp),
            "wts": wts_np,
        })
    return in_maps


def _head(percore, conv_b, fc1_w, fc1_b, fc2_w, fc2_b):
    m = percore.max(axis=0).astype(np.float64)
    eeg_o = m[0:80].reshape(10, 8).max(axis=1)
    wavA_o = m[80:200].reshape(10, 12).max(axis=1)
    wavB_o = m[200:320].reshape(10, 12).max(axis=1)
    conv_b = np.asarray(conv_b, np.float64)
    f = np.concatenate([np.maximum(wavA_o + conv_b[0], 0.0),
                        np.maximum(eeg_o + conv_b[1], 0.0),
                        np.maximum(wavB_o + conv_b[2], 0.0)])
    h = 1.0 / (1.0 + np.exp(-(f @ np.asarray(fc1_w, np.float64).T
                              + np.asarray(fc1_b, np.float64))))
    o = 1.0 / (1.0 + np.exp(-(h @ np.asarray(fc2_w, np.float64).T
                              + np.asarray(fc2_b, np.float64))))
    return o[None, :].astype(np.float32)


def _percore_from_out(arr):
    arr = np.asarray(arr, np.float32)
    return np.concatenate([arr[0:80, [0, 1, 4, 5]].max(axis=1),
                           arr[0:120, [2, 6, 7]].max(axis=1),
                           arr[0:120, [3, 8, 9]].max(axis=1)])


def kernel(x, mu, projA_w, projB_w, conv_w, conv_b, fc1_w, fc1_b, fc2_w, fc2_b):
    global LAST_RESULT
    in_maps = _prepare_in_maps(x, mu, projA_w, projB_w, conv_w)
    nc = _get_nc()

    if os.environ.get("KERNEL_USE_SIM"):
        from concourse.bass_interp import CoreSim
        percore = np.zeros((NCORES, 320), np.float32)
        for k in range(NCORES):
            sim = CoreSim(nc)
            for name, arr in in_maps[k].items():
                sim.tensor(name)[:] = arr
            sim.simulate()
            percore[k] = _percore_from_out(sim.tensor("out"))
    else:
        from concourse.bass_utils import run_bass_kernel_spmd
        trace = bool(os.environ.get("KERNEL_TRACE"))
        res = run_bass_kernel_spmd(nc, in_maps, list(range(NCORES)),
                                   trace=trace)
        LAST_RESULT = res
        percore = np.stack([_percore_from_out(res.results[k]["out"])
                            for k in range(NCORES)])

    return _head(percore, conv_b, fc1_w, fc1_b, fc2_w, fc2_b)
